# revision 20
# baseline (speedup 1.0000x reference)
"""DeltaNet forward on 8 Trainium2 NeuronCores.

Sharding: B*H = 2*16 = 32 (batch, head) pairs -> 4 heads per core, one batch
per group of 4 cores (core d: b = d//4, heads 4*(d%4) .. 4*(d%4)+4).

Host<->device traffic is the bottleneck (axon tunnel ~50MB/s), so I/O is
minimized: each core receives only its L-quarter of x, pre-transposed, in
f16 (2MB) plus f16 weight slices; an on-device AllGather over the 4-core
batch group reconstructs the full x^T. Each core computes its heads'
projections (tensor-parallel columns), short causal conv + SiLU, l2 norm,
the chunked DeltaNet recurrence (chunk C=128, WY/Neumann doubling truncated
at N^8), per-head RMSNorm and its slice of the output projection; the 4
partial outputs per batch are summed on-device by a ReduceScatter, so each
core returns only a [1024,1024] f16 slice of the final output.

Math per head (S in R^{64x64}):
  U solves (I + tril_strict(diag(beta) K K^T)) U = diag(beta)(V - K S0)
  via U <- U + N^{2^j} U, N = -tril_strict(...), j = 0..3
  O = Q S0 + triu_incl(K Q^T)^T-applied U ;  S <- S0 + K^T U
"""

import numpy as np

import concourse.bacc as bacc
import concourse.mybir as mybir
import concourse.tile as tile
from concourse.bass import ds, ts
from concourse.masks import make_identity

f32 = mybir.dt.float32
f16 = mybir.dt.float16
u32 = mybir.dt.uint32
AF = mybir.ActivationFunctionType
ALU = mybir.AluOpType

D = 1024
CH = 256          # channels per core (4 heads x 64)
HD = 64
NH = 4            # heads per core
C = 128           # recurrence chunk
NLEV = 4          # Neumann doubling levels (N, N^2, N^4, N^8)
BLK = 512         # L streaming block
LQ = 1024         # L rows per core (L/4)
EPS = 1e-5
MAGIC = 0x5F3759DF
RG = [[0, 1, 2, 3], [4, 5, 6, 7]]      # batch groups (x gather, out reduce)
PRG = [[0, 4], [1, 5], [2, 6], [3, 7]]  # batch-pair groups (weight dedup)


def _newton_rsqrt(nc, pool, s_ap, out_ap, part, width, magic, iters=1):
    """out = rsqrt(s) elementwise. s_ap f32 (SBUF or PSUM), out any dtype."""
    y_u = pool.tile([part, width], u32, tag="nwt_u")
    nc.any.tensor_scalar(y_u[:], s_ap.bitcast(u32), 1, None,
                         ALU.logical_shift_right)
    nc.any.tensor_tensor(y_u[:], magic[0:part, :].broadcast_to([part, width]),
                         y_u[:], ALU.subtract)
    y_f = y_u[:].bitcast(f32)
    t = pool.tile([part, width], f32, tag="nwt_t")
    for it in range(iters):
        nc.any.tensor_tensor(t[:], y_f, y_f, ALU.mult)
        nc.any.tensor_tensor(t[:], t[:], s_ap, ALU.mult)
        nc.any.tensor_scalar(t[:], t[:], -0.5, 1.5, ALU.mult, ALU.add)
        if it == iters - 1:
            nc.any.tensor_tensor(out_ap, y_f, t[:], ALU.mult)
        else:
            nc.any.tensor_tensor(y_f, y_f, t[:], ALU.mult)


def build(L=4096, use_silu=True):
    nc = bacc.Bacc("TRN2", target_bir_lowering=False, debug=False,
                   num_devices=8)
    # x^T quarter: [D, L/4] f16 (this core's group rank owns L rows
    # [1024*g, 1024*(g+1)) of its batch)
    xq_d = nc.dram_tensor("xq", [D, LQ], f16, kind="ExternalInput").ap()
    # packed weights, deduped across the batch pair (cores d and d+4 use
    # identical weights; each ships half, a pair-AllGather reconstructs):
    # cols 0:772 = rows 512d of w [1024,772]; cols 772:1028 = this half's
    # [128,1024] of wo as a [512,256] flat view; cols 1028:1052 rows 0:128 =
    # conv weights interleaved [128, 6*4] f16
    wh_d = nc.dram_tensor("wh", [512, 1052], f16, kind="ExternalInput").ap()
    out_d = nc.dram_tensor("out", [LQ, D], f16, kind="ExternalOutput").ap()

    nblk = L // BLK
    with tile.TileContext(nc) as tc:
        with (
            tc.tile_pool(name="dram", bufs=1, space="DRAM") as dram,
            tc.tile_pool(name="const", bufs=1) as cst,
            tc.tile_pool(name="state", bufs=1) as st,
            tc.tile_pool(name="xt", bufs=9) as xtp,
            tc.tile_pool(name="sil", bufs=7) as silp,
            tc.tile_pool(name="qkt", bufs=2) as qktp,
            tc.tile_pool(name="acc", bufs=2) as accp,
            tc.tile_pool(name="rows", bufs=3) as rowp,
            tc.tile_pool(name="chain", bufs=2) as chp,
            tc.tile_pool(name="atp", bufs=5) as atp,
            tc.tile_pool(name="upool", bufs=3) as up,
            tc.tile_pool(name="small", bufs=2) as smp,
            tc.tile_pool(name="oT", bufs=2) as oTp,
            tc.tile_pool(name="psA", bufs=2, space="PSUM") as psA,
            tc.tile_pool(name="psB", bufs=2, space="PSUM") as psB,
            tc.tile_pool(name="psC", bufs=3, space="PSUM") as psC,
        ):
            # ---------------- collective staging ----------------
            xq_bounce = dram.tile([D, LQ], f16)
            xt_all = dram.tile([4 * D, LQ], f16)   # 4 stacked x^T quarters
            w_bounce = dram.tile([512, 772], f16)
            w_full = dram.tile([D, 772], f16)
            wo_bounce = dram.tile([512, 256], f16)
            wo_full = dram.tile([CH, D], f16)
            oprt = dram.tile([L, D], f16)          # this core's partial out
            ored = dram.tile([LQ, D], f16)         # summed slice after RS

            nc.sync.dma_start(xq_bounce[:], xq_d)
            nc.gpsimd.collective_compute(
                "AllGather", ALU.bypass, replica_groups=RG,
                ins=[xq_bounce[:]], outs=[xt_all[:]])
            nc.sync.dma_start(w_bounce[:], wh_d[:, 0:772])
            nc.gpsimd.collective_compute(
                "AllGather", ALU.bypass, replica_groups=PRG,
                ins=[w_bounce[:]], outs=[w_full[:]])
            nc.sync.dma_start(wo_bounce[:], wh_d[:, 772:1028])
            nc.gpsimd.collective_compute(
                "AllGather", ALU.bypass, replica_groups=PRG,
                ins=[wo_bounce[:]], outs=[wo_full[:]])

            # ---------------- constants ----------------
            ident16 = cst.tile([128, 128], f16)
            make_identity(nc, ident16)
            ident32 = cst.tile([2, 2], f32)
            make_identity(nc, ident32)
            magic = cst.tile([128, 1], u32)
            nc.gpsimd.memset(magic[:], MAGIC)

            # -1 on strict lower triangle, repeated 4x along free dim
            negtril = cst.tile([128, 512], f16)
            nc.gpsimd.memset(negtril[:, 0:128], 0.0)
            nc.gpsimd.affine_select(
                out=negtril[:, 0:128], in_=negtril[:, 0:128],
                compare_op=ALU.is_ge, fill=-1.0, base=0,
                pattern=[[1, 128]], channel_multiplier=-1)
            # 1 on upper triangle (incl diag), repeated 4x
            triu = cst.tile([128, 512], f16)
            nc.gpsimd.memset(triu[:, 0:128], 1.0)
            nc.gpsimd.affine_select(
                out=triu[:, 0:128], in_=triu[:, 0:128],
                compare_op=ALU.is_ge, fill=0.0, base=0,
                pattern=[[1, 128]], channel_multiplier=-1)
            for rep in range(1, 4):
                nc.any.tensor_copy(negtril[:, ts(rep, 128)], negtril[:, 0:128])
                nc.any.tensor_copy(triu[:, ts(rep, 128)], triu[:, 0:128])

            # sumsq lhsT: [128, 2], ones per 64-block
            ones2 = cst.tile([128, 2], f16)
            nc.gpsimd.memset(ones2[:], 0.0)
            nc.gpsimd.memset(ones2[0:64, 0:1], 1.0)
            nc.gpsimd.memset(ones2[64:128, 1:2], 1.0)
            # broadcast map [2, 128] with value 16 (rsqrt scale compensation)
            bm2 = cst.tile([2, 128], f16)
            nc.gpsimd.memset(bm2[:], 16.0)
            nc.gpsimd.affine_select(
                out=bm2[:], in_=bm2[:], compare_op=ALU.is_ge, fill=0.0,
                base=0, pattern=[[1, 128]], channel_multiplier=-64)
            nc.gpsimd.affine_select(
                out=bm2[:], in_=bm2[:], compare_op=ALU.is_ge, fill=0.0,
                base=63, pattern=[[-1, 128]], channel_multiplier=64)

            # ---------------- weights ----------------
            w_sb = []
            for k in range(8):
                t = cst.tile([128, 772], f16, tag=f"w{k}")
                nc.sync.dma_start(t[:], w_full[ts(k, 128), :])
                w_sb.append(t)
            wo_sb = []
            for j in range(2):
                t = cst.tile([128, D], f16, tag=f"wo{j}")
                nc.sync.dma_start(t[:], wo_full[ts(j, 128), :])
                wo_sb.append(t)
            cwp16 = cst.tile([128, 24], f16)
            nc.sync.dma_start(cwp16[:], wh_d[0:128, 1028:1052])
            cwp = cst.tile([128, 24], f32)
            nc.any.tensor_copy(cwp[:], cwp16[:])

            # ---------------- persistent state ----------------
            ring = []
            for m in range(6):
                t = st.tile([128, BLK + 3], f16, tag=f"ring{m}")
                nc.gpsimd.memset(t[:, 0:3], 0.0)
                ring.append(t)
            S32 = st.tile([64, 256], f32)
            nc.gpsimd.memset(S32[:], 0.0)
            S16 = st.tile([64, 256], f16)
            nc.gpsimd.memset(S16[:], 0.0)

            # ---------------- main streaming loop ----------------
            for blk in range(nblk):
                L0 = blk * BLK
                # x^T tiles straight from the AllGathered buffer
                qrow = D * (L0 // LQ)
                qcol = L0 % LQ
                xt = []
                for k in range(8):
                    t = xtp.tile([128, BLK], f16, tag="xt")
                    nc.sync.dma_start(
                        t[:], xt_all[ds(qrow + 128 * k, 128),
                                     ds(qcol, BLK)])
                    xt.append(t)

                # projections (772 cols) + ring update
                sil = []
                for m in range(6):
                    pp = psA.tile([128, BLK], f32, tag="pA")
                    for k in range(8):
                        nc.tensor.matmul(pp[:], w_sb[k][:, ts(m, 128)],
                                         xt[k][:], start=(k == 0),
                                         stop=(k == 7))
                    rg = ring[m]
                    if blk > 0:
                        nc.any.tensor_copy(rg[:, 0:3], rg[:, BLK:BLK + 3])
                    nc.any.tensor_copy(rg[:, 3:BLK + 3], pp[:])
                    # conv (4 taps) in f32 acc
                    a0 = accp.tile([128, BLK], f32, tag="cacc")
                    nc.any.tensor_scalar(a0[:], rg[:, 0:BLK],
                                         cwp[:, 4 * m:4 * m + 1], None,
                                         ALU.mult)
                    for j in range(1, 4):
                        a1 = accp.tile([128, BLK], f32, tag="cacc")
                        nc.vector.scalar_tensor_tensor(
                            a1[:], rg[:, j:BLK + j],
                            cwp[:, 4 * m + j:4 * m + j + 1],
                            a0[:], ALU.mult, ALU.add)
                        a0 = a1
                    s = silp.tile([128, BLK], f16, tag="sil")
                    if use_silu:
                        nc.scalar.activation(s[:], a0[:], AF.Silu)
                    else:  # CoreSim has no Silu; sigmoid * x is identical
                        sg = accp.tile([128, BLK], f16, tag="sg",
                                       name=f"sg_{blk}_{m}")
                        nc.scalar.activation(sg[:], a0[:], AF.Sigmoid)
                        nc.any.tensor_tensor(s[:], a0[:], sg[:], ALU.mult)
                    sil.append(s)

                # beta = sigmoid(x @ wb) via tanh; two [2, BLK] halves
                # (DVE/ACT partition bases must be 0/32/64/96)
                beta = []
                for mi in range(2):
                    pb = psC.tile([2, BLK], f32, tag="pC",
                                  name=f"pb_{blk}_{mi}")
                    cols = ds(768 + 2 * mi, 2)
                    for k in range(8):
                        nc.tensor.matmul(pb[:], w_sb[k][:, cols], xt[k][:],
                                         start=(k == 0), stop=(k == 7))
                    bth = rowp.tile([2, BLK], f32, tag="brow",
                                    name=f"bth_{blk}_{mi}")
                    nc.scalar.activation(bth[:], pb[:], AF.Tanh, scale=0.5)
                    bt2 = rowp.tile([2, BLK], f32, tag="brow",
                                    name=f"beta_{blk}_{mi}")
                    nc.any.tensor_scalar(bt2[:], bth[:], 0.5, 0.5,
                                         ALU.mult, ALU.add)
                    beta.append(bt2)

                # sumsq rows, per 128-partition tile half: [2, BLK] psum
                def sumsq(m0, mi):
                    sq = accp.tile([128, BLK], f16, tag="sq")
                    nc.scalar.activation(sq[:], sil[m0 + mi][:],
                                         AF.Square, scale=16.0)
                    ps = psC.tile([2, BLK], f32, tag="pC")
                    nc.tensor.matmul(ps[:], ones2[:], sq[:],
                                     start=True, stop=True)
                    return ps

                # q: no explicit normalization — |q|^2 folds into the
                # RMSNorm epsilon (rms = rsqrt(mean(o~^2) + eps*|q|^2)).
                sqq_sb = []
                for mi in range(2):
                    ps = sumsq(0, mi)
                    t = rowp.tile([2, BLK], f32, tag="sqq")
                    nc.any.tensor_copy(t[:], ps[:])
                    sqq_sb.append(t)
                # k: khat = k * rsqrt(|k|^2), ktil = k * beta * rsqrt(|k|^2)
                # stored per-head at partition base 0 (base-64 matmul
                # operands hang TRN2)
                khat = [None] * 4
                ktil = [None] * 4
                for mi in range(2):
                    ps = sumsq(2, mi)
                    rs = rowp.tile([2, BLK], f16, tag="rsk")
                    _newton_rsqrt(nc, smp, ps[:], rs[:], 2, BLK, magic)
                    rsb = rowp.tile([2, BLK], f16, tag="rsb")
                    nc.any.tensor_tensor(rsb[:], rs[:], beta[mi][:],
                                         ALU.mult)
                    for rows, outl, tag in ((rs, khat, "kh"), (rsb, ktil, "kt")):
                        pbc = psB.tile([128, BLK], f32, tag="pB")
                        nc.tensor.matmul(pbc[:], bm2[:], rows[:],
                                         start=True, stop=True)
                        for hh in range(2):
                            h = 2 * mi + hh
                            o = qktp.tile([64, BLK], f16, tag=f"{tag}{h}",
                                          name=f"{tag}{h}_{blk}")
                            pr = ds(64 * hh, 64)
                            nc.any.tensor_tensor(o[:], sil[2 + mi][pr, :],
                                                 pbc[pr, :], ALU.mult)
                            outl[h] = o
                # q, v: odd heads copied to base-0 tiles; even heads alias
                qh_t = [None] * 4
                vh_t = [None] * 4
                for mi in range(2):
                    for hh in range(2):
                        h = 2 * mi + hh
                        if hh == 0:
                            qh_t[h] = sil[mi]
                            vh_t[h] = sil[4 + mi]
                        else:
                            tq = qktp.tile([64, BLK], f16, tag=f"qs{h}",
                                           name=f"qs{h}_{blk}")
                            nc.any.tensor_copy(tq[:], sil[mi][ds(64, 64), :])
                            qh_t[h] = tq
                            tv = qktp.tile([64, BLK], f16, tag=f"vs{h}",
                                           name=f"vs{h}_{blk}")
                            nc.any.tensor_copy(tv[:],
                                               sil[4 + mi][ds(64, 64), :])
                            vh_t[h] = tv

                # ---------------- recurrence: 4 chunk-quads ----------------
                for cq in range(BLK // C):
                    psl = ds(C * cq, C)

                    def hs(tl, h):
                        return tl[h][0:64, psl]

                    id64 = ident16[0:64, 0:64]

                    # beta_t [128, 0:4] and |q|^2_t [128, 4:8] (position-major)
                    pbt = psC.tile([128, 8], f32, tag="pC")
                    for src, c0 in ((beta[0], 0), (beta[1], 2),
                                    (sqq_sb[0], 4), (sqq_sb[1], 6)):
                        nc.tensor.matmul(pbt[:, ds(c0, 2)], src[:, psl],
                                         ident32[0:2, 0:2],
                                         start=True, stop=True)
                    bt = smp.tile([128, 8], f32, tag="bt")
                    nc.any.tensor_copy(bt[:], pbt[:])

                    # G' = Ktil K^T (beta-scaled gram), A0 = -tril_strict
                    pg = psA.tile([128, 512], f32, tag="pA")
                    for h in range(NH):
                        nc.tensor.matmul(pg[:, ts(h, 128)], hs(ktil, h),
                                         hs(khat, h), start=True, stop=True)
                    a_j = chp.tile([128, 512], f16, tag="a")
                    nc.any.tensor_tensor(a_j[:], pg[:], negtril[:], ALU.mult)
                    # transposed chain
                    at = []
                    pt = psB.tile([128, 512], f32, tag="pB")
                    for h in range(NH):
                        nc.tensor.matmul(pt[:, ts(h, 128)],
                                         a_j[:, ts(h, 128)], ident16[:],
                                         start=True, stop=True)
                    t = atp.tile([128, 512], f16, tag="at")
                    nc.any.tensor_copy(t[:], pt[:])
                    at.append(t)
                    for lev in range(1, NLEV):
                        pg2 = psA.tile([128, 512], f32, tag="pA")
                        for h in range(NH):
                            nc.tensor.matmul(pg2[:, ts(h, 128)],
                                             at[-1][:, ts(h, 128)],
                                             a_j[:, ts(h, 128)],
                                             start=True, stop=True)
                        a_n = chp.tile([128, 512], f16, tag="a")
                        nc.any.tensor_copy(a_n[:], pg2[:])
                        a_j = a_n
                        pt2 = psB.tile([128, 512], f32, tag="pB")
                        for h in range(NH):
                            nc.tensor.matmul(pt2[:, ts(h, 128)],
                                             a_j[:, ts(h, 128)], ident16[:],
                                             start=True, stop=True)
                        t = atp.tile([128, 512], f16, tag="at")
                        nc.any.tensor_copy(t[:], pt2[:])
                        at.append(t)

                    # v_row, k_row via transposes
                    pv = psC.tile([128, 256], f32, tag="pC")
                    for h in range(NH):
                        nc.tensor.matmul(pv[:, ts(h, 64)],
                                         hs(vh_t, h), id64,
                                         start=True, stop=True)
                    v_row = up.tile([128, 256], f16, tag="vrow")
                    nc.any.tensor_copy(v_row[:], pv[:])
                    pk = psC.tile([128, 256], f32, tag="pC")
                    for h in range(NH):
                        nc.tensor.matmul(pk[:, ts(h, 64)],
                                         hs(khat, h), id64,
                                         start=True, stop=True)
                    k_row = up.tile([128, 256], f16, tag="krow")
                    nc.any.tensor_copy(k_row[:], pk[:])

                    # R = beta*V - Ktil @ S
                    pks = psC.tile([128, 256], f32, tag="pC")
                    for h in range(NH):
                        nc.tensor.matmul(pks[:, ts(h, 64)], hs(ktil, h),
                                         S16[:, ts(h, 64)],
                                         start=True, stop=True)
                    u_j = up.tile([128, 256], f16, tag="u")
                    for h in range(NH):
                        nc.vector.scalar_tensor_tensor(
                            u_j[:, ts(h, 64)], v_row[:, ts(h, 64)],
                            bt[:, h:h + 1], pks[:, ts(h, 64)],
                            ALU.mult, ALU.subtract)

                    # U-chain applies
                    for lev in range(NLEV):
                        pu = psC.tile([128, 256], f32, tag="pC")
                        for h in range(NH):
                            nc.tensor.matmul(pu[:, ts(h, 64)],
                                             at[lev][:, ts(h, 128)],
                                             u_j[:, ts(h, 64)],
                                             start=True, stop=True)
                        u_n = up.tile([128, 256], f16, tag="u")
                        nc.any.tensor_add(u_n[:], u_j[:], pu[:])
                        u_j = u_n

                    # W = triu_incl(K Q^T)
                    pgq = psA.tile([128, 512], f32, tag="pA")
                    for h in range(NH):
                        nc.tensor.matmul(pgq[:, ts(h, 128)], hs(khat, h),
                                         hs(qh_t, h), start=True, stop=True)
                    wt = chp.tile([128, 512], f16, tag="w")
                    nc.any.tensor_tensor(wt[:], pgq[:], triu[:], ALU.mult)

                    # O = Q S + W^T-applied U
                    po = psB.tile([128, 256], f32, tag="pB")
                    for h in range(NH):
                        nc.tensor.matmul(po[:, ts(h, 64)], hs(qh_t, h),
                                         S16[:, ts(h, 64)],
                                         start=True, stop=False)
                        nc.tensor.matmul(po[:, ts(h, 64)],
                                         wt[:, ts(h, 128)],
                                         u_j[:, ts(h, 64)],
                                         start=False, stop=True)

                    # S += K^T U
                    psi = psC.tile([64, 256], f32, tag="pC")
                    for h in range(NH):
                        nc.tensor.matmul(psi[:, ts(h, 64)],
                                         k_row[:, ts(h, 64)],
                                         u_j[:, ts(h, 64)],
                                         start=True, stop=True)
                    nc.any.tensor_add(S32[:], S32[:], psi[:])
                    nc.any.tensor_copy(S16[:], S32[:])

                    # RMSNorm(o) * 8 (o_norm_w == 1)
                    osq = accp.tile([128, 256], f32, tag="osq")
                    nc.scalar.activation(osq[:], po[:], AF.Square)
                    ssq = smp.tile([128, 4], f32, tag="ssq")
                    nc.vector.tensor_reduce(
                        ssq[:].rearrange("p (f o) -> p f o", o=1),
                        osq[:].rearrange("p (g f) -> p g f", g=4),
                        mybir.AxisListType.X, ALU.add)
                    # eps fold: rms = 8*rsqrt(sum(o~^2) + eps*64/256 * sqq')
                    nc.vector.scalar_tensor_tensor(
                        ssq[:], bt[:, 4:8], EPS * 64.0 / 256.0, ssq[:],
                        ALU.mult, ALU.add)
                    rms = smp.tile([128, 4], f32, tag="rms")
                    _newton_rsqrt(nc, smp, ssq[:], rms[:], 128, 4, magic,
                                  iters=2)
                    o_row = up.tile([128, 256], f16, tag="orow")
                    nc.vector.scalar_tensor_tensor(
                        o_row[:].rearrange("p (g f) -> p g f", g=4),
                        po[:].rearrange("p (g f) -> p g f", g=4),
                        8.0,
                        rms[:].rearrange("p (g o) -> p g o", o=1)
                        .broadcast_to([128, 4, 64]),
                        ALU.mult, ALU.mult)

                    # oT tiles
                    if cq == 0:
                        oT = [oTp.tile([128, BLK], f16, tag=f"oT{j}",
                                       name=f"oT{j}_{blk}")
                              for j in range(2)]
                    pot = psC.tile([128, 256], f32, tag="pC")
                    for h in range(NH):
                        nc.tensor.matmul(
                            pot[ds(64 * (h % 2), 64), ds(128 * (h // 2), 128)],
                            o_row[:, ts(h, 64)], ident16[:],
                            start=True, stop=True)
                    nc.any.tensor_copy(oT[0][:, psl], pot[:, 0:128])
                    nc.any.tensor_copy(oT[1][:, psl], pot[:, 128:256])

                # ---------------- output projection (partial) ----------------
                for mo in range(2):
                    for il in range(4):
                        pw = psB.tile([128, 512], f32, tag="pB")
                        nc.tensor.matmul(pw[:], oT[0][:, ts(il, 128)],
                                         wo_sb[0][:, ds(512 * mo, 512)],
                                         start=True, stop=False)
                        nc.tensor.matmul(pw[:], oT[1][:, ts(il, 128)],
                                         wo_sb[1][:, ds(512 * mo, 512)],
                                         start=False, stop=True)
                        ow = accp.tile([128, 512], f16, tag="ow",
                                       name=f"ow_{blk}_{mo}_{il}")
                        nc.any.tensor_copy(ow[:], pw[:])
                        nc.sync.dma_start(
                            oprt[ds(L0 + 128 * il, 128), ds(512 * mo, 512)],
                            ow[:])

            # ---------------- sum partials across the batch group ----------
            nc.gpsimd.collective_compute(
                "ReduceScatter", ALU.add, replica_groups=RG,
                ins=[oprt[:]], outs=[ored[:]])
            nc.sync.dma_start(out_d, ored[:])

    nc.compile()
    return nc


# ---------------------------------------------------------------------------
_NC_CACHE = {}


def _get_nc(L):
    if L not in _NC_CACHE:
        _NC_CACHE[L] = build(L)
    return _NC_CACHE[L]


def device_inputs(inputs, d):
    g = d % 4
    b = d // 4
    hb = d // 4          # which half of the pair-deduped weights to ship
    cs = slice(256 * g, 256 * (g + 1))
    x = np.asarray(inputs["hidden_states"], np.float32)[b]
    xq = np.ascontiguousarray(
        x[1024 * g:1024 * (g + 1), :].T.astype(np.float16))
    w = np.concatenate([
        np.asarray(inputs["Wq"], np.float32)[:, cs],
        np.asarray(inputs["Wk"], np.float32)[:, cs],
        np.asarray(inputs["Wv"], np.float32)[:, cs],
        np.asarray(inputs["Wb"], np.float32)[:, 4 * g:4 * g + 4],
    ], axis=1).astype(np.float16)
    cw = np.concatenate([
        np.asarray(inputs["conv_q"], np.float32)[cs],
        np.asarray(inputs["conv_k"], np.float32)[cs],
        np.asarray(inputs["conv_v"], np.float32)[cs],
    ], axis=0).astype(np.float16)          # [768, 4]
    wo = np.asarray(inputs["Wo"], np.float32)[cs, :].astype(np.float16)
    wh = np.zeros((512, 1052), np.float16)
    wh[:, 0:772] = w[512 * hb:512 * (hb + 1), :]
    wh[:, 772:1028] = wo[128 * hb:128 * (hb + 1), :].reshape(512, 256)
    # conv weights interleaved: cwp[p, 4m+j] = cw[128m+p, j]
    wh[0:128, 1028:1052] = cw.reshape(6, 128, 4).transpose(1, 0, 2)\
        .reshape(128, 24)
    return {"xq": xq, "wh": wh}


_RUNNER_CACHE = {}


def _get_runner(nc, n_cores=8):
    """PJRT runner equivalent to bass_utils.run_bass_kernel_spmd's axon path
    (bass2jax.run_bass_via_pjrt), with two changes: the jitted function is
    built once and reused (run_bass_kernel_spmd re-traces per call), and the
    output-buffer operand is not a freshly shipped zeros array.  The NEFF
    writes every element of "out", so that operand's contents never matter —
    the already-transferred xq device array (same global shape/dtype,
    [8192,1024] f16) stands in for it, with donation disabled so the result
    buffer is allocated fresh on device."""
    import jax
    from jax.sharding import Mesh, PartitionSpec, NamedSharding
    from jax.experimental.shard_map import shard_map
    import concourse.mybir as mybir
    from concourse.bass2jax import (_bass_exec_p, install_neuronx_cc_hook,
                                    partition_id_tensor)

    install_neuronx_cc_hook()
    partition_name = (nc.partition_id_tensor.name
                      if nc.partition_id_tensor else None)
    in_names, out_names, out_avals = [], [], []
    for alloc in nc.m.functions[0].allocations:
        if not isinstance(alloc, mybir.MemoryLocationSet):
            continue
        name = alloc.memorylocations[0].name
        if alloc.kind == "ExternalInput":
            if name != partition_name:
                in_names.append(name)
        elif alloc.kind == "ExternalOutput":
            out_names.append(name)
            out_avals.append(jax.core.ShapedArray(
                tuple(alloc.tensor_shape), mybir.dt.np(alloc.dtype)))
    n_params = len(in_names)
    n_outs = len(out_avals)
    in_names.extend(out_names)
    if partition_name is not None:
        in_names.append(partition_name)
    # the xq input doubles as the dummy out-buffer operand
    assert n_outs == 1 and in_names[0] == "xq"
    assert out_avals[0].shape == (LQ, D) and out_avals[0].dtype == np.float16

    def _body(*args):
        operands = list(args)
        if partition_name is not None:
            operands.append(partition_id_tensor())
        outs = _bass_exec_p.bind(
            *operands, out_avals=tuple(out_avals), in_names=tuple(in_names),
            out_names=tuple(out_names), lowering_input_output_aliases=(),
            sim_require_finite=True, sim_require_nnan=True, nc=nc)
        return tuple(outs)

    devices = jax.devices()[:n_cores]
    mesh = Mesh(np.asarray(devices), ("core",))
    sh = NamedSharding(mesh, PartitionSpec("core"))
    sharded = jax.jit(
        shard_map(_body, mesh=mesh,
                  in_specs=(PartitionSpec("core"),) * (n_params + n_outs),
                  out_specs=(PartitionSpec("core"),) * n_outs,
                  check_rep=False),
        keep_unused=True)

    def run(concat_in):
        from concurrent.futures import ThreadPoolExecutor
        # async device_put enqueues the transfers; the jit compile on the
        # first call proceeds while they stream
        dev_in = [jax.device_put(a, sh) for a in concat_in]
        out_arrs = sharded(dev_in[0], *dev_in[1:], dev_in[0])
        # fetch the 8 output shards concurrently — the tunnel pipelines
        # better than one big serial pull of the global array
        shards = sorted(out_arrs[0].addressable_shards,
                        key=lambda s: s.index[0].start or 0)
        with ThreadPoolExecutor(n_cores) as ex:
            datas = list(ex.map(lambda s: np.asarray(s.data), shards))
        return [{out_names[0]: datas[c]} for c in range(n_cores)]

    return run


def _run_resilient(nc, L, concat_in):
    """Run via the cached jit runner; on a transient axon-tunnel failure
    (mesh desync / worker hang-up) retry, then fall back to the stock
    run_bass_kernel_spmd path."""
    import time
    try:
        if L not in _RUNNER_CACHE:
            _RUNNER_CACHE[L] = _get_runner(nc)
        return _RUNNER_CACHE[L](concat_in)
    except Exception:
        pass
    time.sleep(20)
    try:
        return _RUNNER_CACHE[L](concat_in)
    except Exception:
        pass
    time.sleep(20)
    from concourse.bass_utils import run_bass_kernel_spmd
    in_maps = [{"xq": concat_in[0][D * d:D * (d + 1)],
                "wh": concat_in[1][512 * d:512 * (d + 1)]}
               for d in range(8)]
    return run_bass_kernel_spmd(nc, in_maps, core_ids=list(range(8))).results


def kernel(**inputs):
    from concurrent.futures import ThreadPoolExecutor
    L = np.asarray(inputs["hidden_states"]).shape[1]
    nc = _get_nc(L)
    # prep per-core shards straight into the global sharded buffers
    xq_g = np.empty((8 * D, LQ), np.float16)
    wh_g = np.empty((8 * 512, 1052), np.float16)

    def prep(d):
        m = device_inputs(inputs, d)
        xq_g[D * d:D * (d + 1)] = m["xq"]
        wh_g[512 * d:512 * (d + 1)] = m["wh"]

    with ThreadPoolExecutor(8) as ex:
        list(ex.map(prep, range(8)))
    results = _run_resilient(nc, L, [xq_g, wh_g])
    out = np.empty((2, L, D), np.float32)
    for b in range(2):
        out[b] = np.concatenate(
            [results[4 * b + g]["out"] for g in range(4)],
            axis=0).astype(np.float32)
    return out


def _warmup(L=4096):
    """Build the Bass module, compile/load the NEFF and prime the axon
    transfer pipeline once at import time with dummy-valued inputs, so
    kernel() calls run at steady state.  Benign constants (not zeros) keep
    the l2-norm rsqrt away from inf on the dummy pass."""
    try:
        nc = _get_nc(L)
        _run_resilient(nc, L, [np.full((8 * D, LQ), 0.01, np.float16),
                               np.full((8 * 512, 1052), 0.01, np.float16)])
    except Exception:
        pass  # never let warming break the import; kernel() retries anyway


_warmup()
